# revision 8
# baseline (speedup 1.0000x reference)
"""Trainium2 Bass kernel for nn_BinaryLoss (BCE triangle-mesh loss).

Strategy
--------
Host side (integer combinatorics on tiny index tensors only):
  * build the unique sorted-triangle key table, the undirected GT edge set,
    per-vertex unique-triangle counts, the candidate-triple membership mask
    gt_mask [N,256], the manifold row weights w [N], and gm [N,16].
  * Two exact identities let the device math stay fully dense:
      - gt_labels_masked == gt_mask  (a GT triangle always contributes its
        own (e0,e1) edge to full_mat, so the full_mat factor is redundant)
      - softplus(x) - x*mask needs only row-sums of softplus(x) and the sum
        of x over masked positions (max 8 per row for this data).
Device side (all floating-point math, data-parallel over 8 cores):
  * rows sharded 2048/core; Softplus+accumulate on ScalarE gives per-row
    softplus sums; masked-x row sums via narrow gathered [rows,8] reduce;
    manifold-weighted combine via fused tensor_tensor_reduce.
  * hard-negative mining: gm==1 groups (16 logits each) are compacted and
    processed with the DVE Max8 instruction (exact top-8, descending) ->
    ranks 1,2 are pos/neg; Softplus(scale=-1)+accumulate folds the
    BCE-vs-ones/zeros sums. Padding groups [B,B,-B,...] contribute ~1e-13.
  * per-core partials reduced across partitions with a ones-matmul on PE;
    the 8 per-core partial triples are summed on host (scalar all-reduce).
"""
import os
import numpy as np

N_CORES = 8
B_PAD = 30.0  # pad-group magnitude: softplus(-30) ~ 9e-14


# ---------------------------------------------------------------- host prep
def _host_prep(pred_logits, points, knn_indices, gt_triangles):
    N, K = knn_indices.shape
    M = (K - 1) * (K - 1)
    num_pts = points.shape[0]
    P = num_pts + 1

    tri = np.sort(np.asarray(gt_triangles, dtype=np.int64), axis=1)
    keys = tri[:, 0] * (P * P) + tri[:, 1] * P + tri[:, 2]
    uk = np.unique(keys)

    ut0, ut1, ut2 = uk // (P * P), (uk // P) % P, uk % P
    counts = np.zeros(P, np.float64)
    np.add.at(counts, ut0, 1.0)
    np.add.at(counts, ut1, (ut1 != ut0).astype(np.float64))
    np.add.at(counts, ut2, (ut2 != ut1).astype(np.float64))
    all_N_gt = counts[np.asarray(knn_indices[:, 0], dtype=np.int64)]

    e_u = np.concatenate([np.minimum(tri[:, 0], tri[:, 1]),
                          np.minimum(tri[:, 1], tri[:, 2]),
                          np.minimum(tri[:, 0], tri[:, 2])])
    e_v = np.concatenate([np.maximum(tri[:, 0], tri[:, 1]),
                          np.maximum(tri[:, 1], tri[:, 2]),
                          np.maximum(tri[:, 0], tri[:, 2])])
    ekeys = np.unique(e_u * P + e_v)

    c = np.asarray(knn_indices[:, 0], dtype=np.int64)[:, None]
    a = np.asarray(knn_indices[:, 1:], dtype=np.int64)
    q = np.minimum(c, a) * P + np.maximum(c, a)
    pos = np.clip(np.searchsorted(ekeys, q.ravel()), 0, len(ekeys) - 1)
    gm = (ekeys[pos] == q.ravel()).reshape(N, K - 1)

    e0 = np.repeat(a, K - 1, axis=1)
    e1 = np.tile(a, (1, K - 1))
    v0 = np.broadcast_to(c, e0.shape)
    cand = np.stack([v0, e0, e1], axis=-1)
    cand.sort(axis=-1)
    ck = cand[..., 0] * (P * P) + cand[..., 1] * P + cand[..., 2]
    cpos = np.clip(np.searchsorted(uk, ck.ravel()), 0, len(uk) - 1)
    gt_mask = (uk[cpos] == ck.ravel()).reshape(N, M)

    all_N_pred = gt_mask.sum(1).astype(np.float64)
    manifold = (all_N_gt * 2.0) == all_N_pred
    w = manifold.astype(np.float32)

    inv_denom = np.float32(1.0 / max(float(w.sum(dtype=np.float64)) * M, 1.0))
    inv_cnt = np.float32(1.0 / max(float(gm.sum(dtype=np.float64)), 1.0))
    return gt_mask, gm, w, inv_denom, inv_cnt


def _make_shards(x, gt_mask, gm, w, inv_denom, inv_cnt):
    """Build per-core input dicts. x is [N,256] f32."""
    N, M = x.shape
    rows_per_core = N // N_CORES          # 2048
    parts = 128
    rpp = rows_per_core // parts          # 16 rows per partition

    # masked-x values padded to L per row (L chosen from data, power-of-2-ish)
    mask_per_row = gt_mask.sum(1)
    L = max(8, int(mask_per_row.max()))
    L = int(2 ** np.ceil(np.log2(L)))
    rr, cc = np.nonzero(gt_mask)
    xm = np.zeros((N, L), np.float32)
    slot = np.zeros(N, np.int64)
    # vectorized bucket fill: rank of each nonzero within its row
    row_starts = np.zeros(N + 1, np.int64)
    np.add.at(row_starts, rr + 1, 1)
    row_starts = np.cumsum(row_starts)
    ranks = np.arange(len(rr)) - row_starts[rr]
    xm[rr, ranks] = x[rr, cc]

    # compacted gm groups, padded; distributed evenly over cores
    gn, gi = np.nonzero(gm)               # group ids (row, i)
    total = len(gn)
    per_core = int(np.ceil(total / N_CORES))
    g_chunks = int(np.ceil(per_core / parts))   # free-dim group chunks
    cap = g_chunks * parts                       # groups per core
    pl3 = x.reshape(N, 16, 16)

    pad_group = np.full(16, -B_PAD, np.float32)
    pad_group[0] = B_PAD
    pad_group[1] = B_PAD

    in_maps = []
    for core in range(N_CORES):
        r0, r1 = core * rows_per_core, (core + 1) * rows_per_core
        # transposed: m on partitions (2 x 128), rows on free dim
        xc = np.ascontiguousarray(x[r0:r1].T)                      # [256, 2048]
        # row r = blk*128 + p  ->  (partition p, block blk)
        xmc = np.ascontiguousarray(
            xm[r0:r1].reshape(rpp, parts, L).transpose(1, 0, 2)
        ).reshape(parts, rpp * L)
        wc = np.ascontiguousarray(w[r0:r1].reshape(rpp, parts).T)  # [128, 16]

        lo, hi = core * per_core, min((core + 1) * per_core, total)
        gsel = np.broadcast_to(pad_group, (cap, 16)).copy()
        if hi > lo:
            gsel[: hi - lo] = pl3[gn[lo:hi], gi[lo:hi], :]
        # layout [parts, g_chunks, 16]: group j -> partition j % parts
        gsel = np.ascontiguousarray(
            gsel.reshape(g_chunks, parts, 16).transpose(1, 0, 2)
        ).reshape(parts, g_chunks * 16)

        consts = np.broadcast_to(
            np.array([inv_denom, inv_cnt], np.float32), (parts, 2)
        ).copy()
        in_maps.append({
            "x": xc, "xm": xmc, "wrow": wc, "gsel": gsel, "consts": consts,
        })
    return in_maps, L, g_chunks


# ---------------------------------------------------------------- bass build
def _build_bass(L, g_chunks):
    from contextlib import ExitStack

    import concourse.bacc as bacc
    import concourse.bass as bass
    import concourse.mybir as mybir
    import concourse.tile as tile

    f32 = mybir.dt.float32
    AFT = mybir.ActivationFunctionType
    ALU = mybir.AluOpType
    AX = mybir.AxisListType

    parts, rpp, M = 128, 16, 256
    G = g_chunks

    R = 2048  # rows per core, free dim of transposed tiles

    nc = bacc.Bacc(
        "TRN2", target_bir_lowering=False, debug=False,
        enable_asserts=False, num_devices=N_CORES,
    )
    x_d = nc.dram_tensor("x", [2 * parts, R], f32, kind="ExternalInput").ap()
    xm_d = nc.dram_tensor("xm", [parts, rpp * L], f32, kind="ExternalInput").ap()
    w_d = nc.dram_tensor("wrow", [parts, rpp], f32, kind="ExternalInput").ap()
    g_d = nc.dram_tensor("gsel", [parts, G * 16], f32, kind="ExternalInput").ap()
    c_d = nc.dram_tensor("consts", [parts, 2], f32, kind="ExternalInput").ap()
    out_d = nc.dram_tensor("out", [1, 3], f32, kind="ExternalOutput").ap()

    with tile.TileContext(nc) as tc, ExitStack() as ctx:
        pool = ctx.enter_context(tc.tile_pool(name="main", bufs=1))
        ps_pool = ctx.enter_context(tc.tile_pool(name="ps", bufs=1, space="PSUM"))

        # --- load x^T as two half tiles (m=0..127 / m=128..255) ---
        n_dma = 2
        chunk = R // n_dma
        halves = []
        for h in range(2):
            xth = pool.tile([parts, R], f32, name=f"xt{h}")
            for i in range(n_dma):
                nc.sync.dma_start(
                    xth[:, i * chunk:(i + 1) * chunk],
                    x_d[h * parts:(h + 1) * parts, i * chunk:(i + 1) * chunk])
            halves.append(xth)
        gt = pool.tile([parts, G * 16], f32)
        nc.sync.dma_start(gt[:], g_d[:])
        xmt = pool.tile([parts, rpp * L], f32)
        nc.sync.dma_start(xmt[:], xm_d[:])
        wt = pool.tile([parts, rpp], f32)
        nc.sync.dma_start(wt[:], w_d[:])
        ct = pool.tile([parts, 2], f32)
        nc.sync.dma_start(ct[:], c_d[:])

        ones1 = pool.tile([parts, 1], f32)
        nc.vector.memset(ones1[:], 1.0)

        # --- softplus via Exp then Ln(e+1) on ScalarE; row sums on PE ---
        sps = []
        for h, xth in enumerate(halves):
            eh = pool.tile([parts, R], f32, name=f"e{h}")
            nc.scalar.activation(eh[:], xth[:], AFT.Exp)
            sph = pool.tile([parts, R], f32, name=f"sp{h}")
            nc.scalar.activation(sph[:], eh[:], AFT.Ln, bias=1.0)
            sps.append(sph)

        psum_rs = ps_pool.tile([parts, rpp], f32)
        for blk in range(rpp):
            for h, sph in enumerate(sps):
                nc.tensor.matmul(
                    psum_rs[:, blk:blk + 1],
                    sph[:, blk * parts:(blk + 1) * parts], ones1[:],
                    start=(h == 0), stop=(h == 1),
                )

        # --- masked-x row sums (DVE reduce over narrow gather) ---
        rowxm = pool.tile([parts, rpp], f32)
        nc.vector.tensor_reduce(
            rowxm[:], xmt[:].rearrange("p (r l) -> p r l", l=L),
            axis=AX.X, op=ALU.add,
        )

        # --- weighted main-loss partial: sum_r w_r*(rowsp_r - rowxm_r) ---
        d_t = pool.tile([parts, rpp], f32)
        nc.vector.tensor_tensor(d_t[:], psum_rs[:], rowxm[:], op=ALU.subtract)
        wd = pool.tile([parts, rpp], f32)
        nc.vector.tensor_tensor(wd[:], d_t[:], wt[:], op=ALU.mult)
        mainacc = pool.tile([parts, 1], f32)
        nc.vector.tensor_reduce(mainacc[:], wd[:], axis=AX.X, op=ALU.add)

        # --- top-8 per compacted gm-group (exact, descending) ---
        top8 = pool.tile([parts, G * 8], f32)
        for g in range(G):
            nc.vector.max(top8[:, g * 8:(g + 1) * 8], gt[:, g * 16:(g + 1) * 16])
        t3 = top8[:].rearrange("p (g e) -> p g e", e=8)

        posacc = pool.tile([parts, 1], f32)
        pexp = pool.tile([parts, G], f32)
        nc.scalar.activation(pexp[:], t3[:, :, 1], AFT.Exp, scale=-1.0)
        pln = pool.tile([parts, G], f32)
        nc.scalar.activation(pln[:], pexp[:], AFT.Ln, bias=1.0,
                             accum_out=posacc[:])
        negacc = pool.tile([parts, 1], f32)
        nexp = pool.tile([parts, G], f32)
        nc.scalar.activation(nexp[:], t3[:, :, 2], AFT.Exp)
        nln = pool.tile([parts, G], f32)
        nc.scalar.activation(nln[:], nexp[:], AFT.Ln, bias=1.0,
                             accum_out=negacc[:])

        # --- scale by host constants, pack, partition-reduce via PE ---
        parts_t = pool.tile([parts, 3], f32)
        nc.vector.tensor_tensor(parts_t[:, 0:1], mainacc[:], ct[:, 0:1], op=ALU.mult)
        nc.vector.tensor_tensor(parts_t[:, 1:2], posacc[:], ct[:, 1:2], op=ALU.mult)
        nc.vector.tensor_tensor(parts_t[:, 2:3], negacc[:], ct[:, 1:2], op=ALU.mult)

        ones = pool.tile([parts, 1], f32)
        nc.vector.memset(ones[:], 1.0)
        pst = ps_pool.tile([1, 3], f32)
        nc.tensor.matmul(pst[:], ones[:], parts_t[:], start=True, stop=True)
        sb_out = pool.tile([1, 3], f32)
        nc.scalar.copy(sb_out[:], pst[:])
        nc.sync.dma_start(out_d[:], sb_out[:])

    nc.compile()
    return nc


# ---------------------------------------------------------------- entrypoint
def _run(pred_logits, points, knn_indices, gt_triangles, **run_kwargs):
    from concourse.bass_utils import run_bass_kernel_spmd

    x = np.ascontiguousarray(np.asarray(pred_logits, dtype=np.float32))
    gt_mask, gm, w, inv_denom, inv_cnt = _host_prep(
        pred_logits, points, knn_indices, gt_triangles)
    in_maps, L, g_chunks = _make_shards(x, gt_mask, gm, w, inv_denom, inv_cnt)
    nc = _build_bass(L, g_chunks)
    res = run_bass_kernel_spmd(nc, in_maps, core_ids=list(range(N_CORES)),
                               **run_kwargs)
    total = np.zeros(3, np.float64)
    for r in res.results:
        total += np.asarray(r["out"], dtype=np.float64).reshape(3)
    return total.astype(np.float32), res


def kernel(pred_logits, points, knn_indices, gt_triangles):
    out, _ = _run(pred_logits, points, knn_indices, gt_triangles)
    return out


# revision 12
# speedup vs baseline: 1.1138x; 1.1138x over previous
"""Trainium2 Bass kernel for nn_BinaryLoss (BCE triangle-mesh loss).

Strategy
--------
Host side (integer combinatorics on tiny index tensors only):
  * build the unique sorted-triangle key table, the undirected GT edge set,
    per-vertex unique-triangle counts, the candidate-triple membership mask
    gt_mask [N,256], the manifold row weights w [N], and gm [N,16].
  * Two exact identities let the device math stay fully dense:
      - gt_labels_masked == gt_mask  (a GT triangle always contributes its
        own (e0,e1) edge to full_mat, so the full_mat factor is redundant)
      - softplus(x) - x*mask needs only row-sums of softplus(x) and the sum
        of x over masked positions (max 8 per row for this data).
Device side (all floating-point math, data-parallel over 8 cores):
  * rows sharded 2048/core; Softplus+accumulate on ScalarE gives per-row
    softplus sums; masked-x row sums via narrow gathered [rows,8] reduce;
    manifold-weighted combine via fused tensor_tensor_reduce.
  * hard-negative mining: gm==1 groups (16 logits each) are compacted and
    processed with the DVE Max8 instruction (exact top-8, descending) ->
    ranks 1,2 are pos/neg; Softplus(scale=-1)+accumulate folds the
    BCE-vs-ones/zeros sums. Padding groups [B,B,-B,...] contribute ~1e-13.
  * per-core partials reduced across partitions with a ones-matmul on PE;
    the 8 per-core partial triples are summed on host (scalar all-reduce).
"""
import os
import numpy as np

N_CORES = 8
B_PAD = 30.0  # pad-group magnitude: softplus(-30) ~ 9e-14


# ---------------------------------------------------------------- host prep
def _host_prep(pred_logits, points, knn_indices, gt_triangles):
    N, K = knn_indices.shape
    M = (K - 1) * (K - 1)
    num_pts = points.shape[0]
    P = num_pts + 1

    tri = np.sort(np.asarray(gt_triangles, dtype=np.int64), axis=1)
    keys = tri[:, 0] * (P * P) + tri[:, 1] * P + tri[:, 2]
    uk = np.unique(keys)

    ut0, ut1, ut2 = uk // (P * P), (uk // P) % P, uk % P
    counts = np.zeros(P, np.float64)
    np.add.at(counts, ut0, 1.0)
    np.add.at(counts, ut1, (ut1 != ut0).astype(np.float64))
    np.add.at(counts, ut2, (ut2 != ut1).astype(np.float64))
    all_N_gt = counts[np.asarray(knn_indices[:, 0], dtype=np.int64)]

    e_u = np.concatenate([np.minimum(tri[:, 0], tri[:, 1]),
                          np.minimum(tri[:, 1], tri[:, 2]),
                          np.minimum(tri[:, 0], tri[:, 2])])
    e_v = np.concatenate([np.maximum(tri[:, 0], tri[:, 1]),
                          np.maximum(tri[:, 1], tri[:, 2]),
                          np.maximum(tri[:, 0], tri[:, 2])])
    ekeys = np.unique(e_u * P + e_v)

    c = np.asarray(knn_indices[:, 0], dtype=np.int64)[:, None]
    a = np.asarray(knn_indices[:, 1:], dtype=np.int64)
    q = np.minimum(c, a) * P + np.maximum(c, a)
    pos = np.clip(np.searchsorted(ekeys, q.ravel()), 0, len(ekeys) - 1)
    gm = (ekeys[pos] == q.ravel()).reshape(N, K - 1)

    e0 = np.repeat(a, K - 1, axis=1)
    e1 = np.tile(a, (1, K - 1))
    v0 = np.broadcast_to(c, e0.shape)
    cand = np.stack([v0, e0, e1], axis=-1)
    cand.sort(axis=-1)
    ck = cand[..., 0] * (P * P) + cand[..., 1] * P + cand[..., 2]
    cpos = np.clip(np.searchsorted(uk, ck.ravel()), 0, len(uk) - 1)
    gt_mask = (uk[cpos] == ck.ravel()).reshape(N, M)

    all_N_pred = gt_mask.sum(1).astype(np.float64)
    manifold = (all_N_gt * 2.0) == all_N_pred
    w = manifold.astype(np.float32)

    inv_denom = np.float32(1.0 / max(float(w.sum(dtype=np.float64)) * M, 1.0))
    inv_cnt = np.float32(1.0 / max(float(gm.sum(dtype=np.float64)), 1.0))
    return gt_mask, gm, w, inv_denom, inv_cnt


def _make_shards(x, gt_mask, gm, w, inv_denom, inv_cnt):
    """Build per-core input dicts. x is [N,256] f32."""
    N, M = x.shape
    rows_per_core = N // N_CORES          # 2048
    parts = 128
    rpp = rows_per_core // parts          # 16 rows per partition

    # masked-x values padded to L per row (L chosen from data, power-of-2-ish)
    mask_per_row = gt_mask.sum(1)
    L = max(8, int(mask_per_row.max()))
    L = int(2 ** np.ceil(np.log2(L)))
    rr, cc = np.nonzero(gt_mask)
    xm = np.zeros((N, L), np.float32)
    slot = np.zeros(N, np.int64)
    # vectorized bucket fill: rank of each nonzero within its row
    row_starts = np.zeros(N + 1, np.int64)
    np.add.at(row_starts, rr + 1, 1)
    row_starts = np.cumsum(row_starts)
    ranks = np.arange(len(rr)) - row_starts[rr]
    xm[rr, ranks] = x[rr, cc]

    # compacted gm groups, padded; distributed evenly over cores
    gn, gi = np.nonzero(gm)               # group ids (row, i)
    total = len(gn)
    per_core = int(np.ceil(total / N_CORES))
    g_chunks = int(np.ceil(per_core / parts))   # free-dim group chunks
    cap = g_chunks * parts                       # groups per core
    pl3 = x.reshape(N, 16, 16)

    pad_group = np.full(16, -B_PAD, np.float32)
    pad_group[0] = B_PAD
    pad_group[1] = B_PAD

    in_maps = []
    for core in range(N_CORES):
        r0, r1 = core * rows_per_core, (core + 1) * rows_per_core
        # transposed: m on partitions (2 x 128), rows on free dim
        xc = np.ascontiguousarray(x[r0:r1].T)                      # [256, 2048]
        # row r -> (partition r//16, slot r%16) to match the PSUM relayout DMA
        xmc = np.ascontiguousarray(xm[r0:r1]).reshape(parts, rpp * L)
        wc = np.ascontiguousarray(w[r0:r1]).reshape(parts, rpp)

        lo, hi = core * per_core, min((core + 1) * per_core, total)
        gsel = np.broadcast_to(pad_group, (cap, 16)).copy()
        if hi > lo:
            gsel[: hi - lo] = pl3[gn[lo:hi], gi[lo:hi], :]
        # layout [parts, g_chunks, 16]: group j -> partition j % parts
        gsel = np.ascontiguousarray(
            gsel.reshape(g_chunks, parts, 16).transpose(1, 0, 2)
        ).reshape(parts, g_chunks * 16)

        consts = np.broadcast_to(
            np.array([inv_denom, inv_cnt], np.float32), (parts, 2)
        ).copy()
        in_maps.append({
            "x": xc, "xm": xmc, "wrow": wc, "gsel": gsel, "consts": consts,
        })
    return in_maps, L, g_chunks


# ---------------------------------------------------------------- bass build
def _build_bass(L, g_chunks):
    from contextlib import ExitStack

    import concourse.bacc as bacc
    import concourse.bass as bass
    import concourse.mybir as mybir
    import concourse.tile as tile

    f32 = mybir.dt.float32
    AFT = mybir.ActivationFunctionType
    ALU = mybir.AluOpType
    AX = mybir.AxisListType

    parts, rpp, M = 128, 16, 256
    G = g_chunks

    R = 2048  # rows per core, free dim of transposed tiles

    nc = bacc.Bacc(
        "TRN2", target_bir_lowering=False, debug=False,
        enable_asserts=False, num_devices=N_CORES,
    )
    x_d = nc.dram_tensor("x", [2 * parts, R], f32, kind="ExternalInput").ap()
    xm_d = nc.dram_tensor("xm", [parts, rpp * L], f32, kind="ExternalInput").ap()
    w_d = nc.dram_tensor("wrow", [parts, rpp], f32, kind="ExternalInput").ap()
    g_d = nc.dram_tensor("gsel", [parts, G * 16], f32, kind="ExternalInput").ap()
    c_d = nc.dram_tensor("consts", [parts, 2], f32, kind="ExternalInput").ap()
    out_d = nc.dram_tensor("out", [1, 3], f32, kind="ExternalOutput").ap()

    bf16 = mybir.dt.bfloat16
    with tile.TileContext(nc) as tc, ExitStack() as ctx:
        pool = ctx.enter_context(tc.tile_pool(name="main", bufs=1))
        ps_pool = ctx.enter_context(tc.tile_pool(name="ps", bufs=1, space="PSUM"))

        # --- load x^T as two half tiles (m=0..127 / m=128..255) ---
        halves = []
        for h in range(2):
            xth = pool.tile([parts, R], f32, name=f"xt{h}")
            nc.sync.dma_start(xth[:], x_d[h * parts:(h + 1) * parts, :])
            halves.append(xth)
        gt = pool.tile([parts, G * 16], f32)
        nc.sync.dma_start(gt[:], g_d[:])
        xmt = pool.tile([parts, rpp * L], f32)
        nc.sync.dma_start(xmt[:], xm_d[:])
        wt = pool.tile([parts, rpp], f32)
        nc.sync.dma_start(wt[:], w_d[:])
        ct = pool.tile([parts, 2], f32)
        nc.sync.dma_start(ct[:], c_d[:])

        onesw = pool.tile([parts, rpp], bf16)  # stationary ones for colsum
        nc.vector.memset(onesw[:], 1.0)

        # --- top-8 per compacted gm-group (exact, descending) ---
        top8 = pool.tile([parts, G * 8], f32)
        for g in range(G):
            nc.vector.max(top8[:, g * 8:(g + 1) * 8], gt[:, g * 16:(g + 1) * 16])
        t3 = top8[:].rearrange("p (g e) -> p g e", e=8)

        # --- softplus: Exp then Ln(e+1); sp in bf16 for the PE reduction ---
        # (ACT program order groups all Exp then all Ln to minimize table loads)
        exps = []
        for h, xth in enumerate(halves):
            eh = pool.tile([parts, R], f32, name=f"e{h}")
            nc.scalar.activation(eh[:], xth[:], AFT.Exp)
            exps.append(eh)
        pn_exp = pool.tile([parts, 2 * G], f32)
        pn_in = t3[:, :, 1:3]  # [128, G, 2] ranks 1 (pos) and 2 (neg)
        nc.scalar.activation(pn_exp[:].rearrange("p (g e) -> p g e", e=2),
                             pn_in, AFT.Exp)
        sps = []
        for h, eh in enumerate(exps):
            sph = pool.tile([parts, R], bf16, name=f"sp{h}")
            nc.scalar.activation(sph[:], eh[:], AFT.Ln, bias=1.0)
            sps.append(sph)
        pn_ln = pool.tile([parts, 2 * G], f32)
        nc.scalar.activation(pn_ln[:], pn_exp[:], AFT.Ln, bias=1.0)

        # --- row sums of softplus: ones^T @ sp on PE (column sums) ---
        psum_cs = ps_pool.tile([rpp, R], f32)
        NCH = 512
        for c in range(R // NCH):
            for h, sph in enumerate(sps):
                nc.tensor.matmul(
                    psum_cs[:, c * NCH:(c + 1) * NCH],
                    onesw[:], sph[:, c * NCH:(c + 1) * NCH],
                    start=(h == 0), stop=(h == 1),
                )
        # relayout row sums: psum row 0 [1, 2048] -> [128, 16] (row r = 16p+j)
        cs1 = pool.tile([1, R], f32)
        nc.scalar.copy(cs1[:], psum_cs[0:1, :])
        rs_sb = pool.tile([parts, rpp], f32)
        nc.sync.dma_start(rs_sb[:], cs1[:])

        # --- masked-x row sums (DVE reduce over narrow gather) ---
        rowxm = pool.tile([parts, rpp], f32)
        nc.vector.tensor_reduce(
            rowxm[:], xmt[:].rearrange("p (r l) -> p r l", l=L),
            axis=AX.X, op=ALU.add,
        )

        # --- weighted main-loss partial: sum_r w_r*(rowsp_r - rowxm_r) ---
        d_t = pool.tile([parts, rpp], f32)
        nc.vector.tensor_tensor(d_t[:], rs_sb[:], rowxm[:], op=ALU.subtract)
        wd = pool.tile([parts, rpp], f32)
        nc.vector.tensor_tensor(wd[:], d_t[:], wt[:], op=ALU.mult)
        mainacc = pool.tile([parts, 1], f32)
        nc.vector.tensor_reduce(mainacc[:], wd[:], axis=AX.X, op=ALU.add)

        # --- pos/neg partials: sp(-pos) = sp(pos) - pos; sp(neg) direct ---
        pn3 = pn_ln[:].rearrange("p (g e) -> p g e", e=2)
        possub = pool.tile([parts, G], f32)
        nc.vector.tensor_tensor(possub[:], pn3[:, :, 0], t3[:, :, 1],
                                op=ALU.subtract)
        posacc = pool.tile([parts, 1], f32)
        nc.vector.tensor_reduce(posacc[:], possub[:], axis=AX.X, op=ALU.add)
        negacc = pool.tile([parts, 1], f32)
        nc.vector.tensor_reduce(negacc[:], pn3[:, :, 1], axis=AX.X, op=ALU.add)

        # --- scale by host constants, pack, partition-reduce via PE ---
        parts_t = pool.tile([parts, 3], f32)
        nc.vector.tensor_tensor(parts_t[:, 0:1], mainacc[:], ct[:, 0:1], op=ALU.mult)
        nc.vector.tensor_tensor(parts_t[:, 1:2], posacc[:], ct[:, 1:2], op=ALU.mult)
        nc.vector.tensor_tensor(parts_t[:, 2:3], negacc[:], ct[:, 1:2], op=ALU.mult)

        ones = pool.tile([parts, 1], f32)
        nc.vector.memset(ones[:], 1.0)
        pst = ps_pool.tile([1, 3], f32)
        nc.tensor.matmul(pst[:], ones[:], parts_t[:], start=True, stop=True)
        sb_out = pool.tile([1, 3], f32)
        nc.scalar.copy(sb_out[:], pst[:])
        nc.sync.dma_start(out_d[:], sb_out[:])

    nc.compile()
    return nc


# ---------------------------------------------------------------- entrypoint
def _run(pred_logits, points, knn_indices, gt_triangles, **run_kwargs):
    from concourse.bass_utils import run_bass_kernel_spmd

    x = np.ascontiguousarray(np.asarray(pred_logits, dtype=np.float32))
    gt_mask, gm, w, inv_denom, inv_cnt = _host_prep(
        pred_logits, points, knn_indices, gt_triangles)
    in_maps, L, g_chunks = _make_shards(x, gt_mask, gm, w, inv_denom, inv_cnt)
    nc = _build_bass(L, g_chunks)
    res = run_bass_kernel_spmd(nc, in_maps, core_ids=list(range(N_CORES)),
                               **run_kwargs)
    total = np.zeros(3, np.float64)
    for r in res.results:
        total += np.asarray(r["out"], dtype=np.float64).reshape(3)
    return total.astype(np.float32), res


def kernel(pred_logits, points, knn_indices, gt_triangles):
    out, _ = _run(pred_logits, points, knn_indices, gt_triangles)
    return out


# revision 16
# speedup vs baseline: 1.1778x; 1.0575x over previous
"""Trainium2 Bass kernel for nn_BinaryLoss (BCE triangle-mesh loss).

Strategy
--------
Host side (integer combinatorics on tiny index tensors only):
  * build the unique sorted-triangle key table, the undirected GT edge set,
    per-vertex unique-triangle counts, the candidate-triple membership mask
    gt_mask [N,256], the manifold row weights w [N], and gm [N,16].
  * Two exact identities let the device math stay fully dense:
      - gt_labels_masked == gt_mask  (a GT triangle always contributes its
        own (e0,e1) edge to full_mat, so the full_mat factor is redundant)
      - softplus(x) - x*mask needs only row-sums of softplus(x) and the sum
        of x over masked positions (max 8 per row for this data).
Device side (all floating-point math, data-parallel over 8 cores):
  * rows sharded 2048/core; Softplus+accumulate on ScalarE gives per-row
    softplus sums; masked-x row sums via narrow gathered [rows,8] reduce;
    manifold-weighted combine via fused tensor_tensor_reduce.
  * hard-negative mining: gm==1 groups (16 logits each) are compacted and
    processed with the DVE Max8 instruction (exact top-8, descending) ->
    ranks 1,2 are pos/neg; Softplus(scale=-1)+accumulate folds the
    BCE-vs-ones/zeros sums. Padding groups [B,B,-B,...] contribute ~1e-13.
  * per-core partials reduced across partitions with a ones-matmul on PE;
    the 8 per-core partial triples are summed on host (scalar all-reduce).
"""
import os
import numpy as np

N_CORES = 8
B_PAD = 30.0  # pad-group magnitude: softplus(-30) ~ 9e-14


# ---------------------------------------------------------------- host prep
def _host_prep(pred_logits, points, knn_indices, gt_triangles):
    N, K = knn_indices.shape
    M = (K - 1) * (K - 1)
    num_pts = points.shape[0]
    P = num_pts + 1

    tri = np.sort(np.asarray(gt_triangles, dtype=np.int64), axis=1)
    keys = tri[:, 0] * (P * P) + tri[:, 1] * P + tri[:, 2]
    uk = np.unique(keys)

    ut0, ut1, ut2 = uk // (P * P), (uk // P) % P, uk % P
    counts = np.zeros(P, np.float64)
    np.add.at(counts, ut0, 1.0)
    np.add.at(counts, ut1, (ut1 != ut0).astype(np.float64))
    np.add.at(counts, ut2, (ut2 != ut1).astype(np.float64))
    all_N_gt = counts[np.asarray(knn_indices[:, 0], dtype=np.int64)]

    e_u = np.concatenate([np.minimum(tri[:, 0], tri[:, 1]),
                          np.minimum(tri[:, 1], tri[:, 2]),
                          np.minimum(tri[:, 0], tri[:, 2])])
    e_v = np.concatenate([np.maximum(tri[:, 0], tri[:, 1]),
                          np.maximum(tri[:, 1], tri[:, 2]),
                          np.maximum(tri[:, 0], tri[:, 2])])
    ekeys = np.unique(e_u * P + e_v)

    c = np.asarray(knn_indices[:, 0], dtype=np.int64)[:, None]
    a = np.asarray(knn_indices[:, 1:], dtype=np.int64)
    q = np.minimum(c, a) * P + np.maximum(c, a)
    pos = np.clip(np.searchsorted(ekeys, q.ravel()), 0, len(ekeys) - 1)
    gm = (ekeys[pos] == q.ravel()).reshape(N, K - 1)

    e0 = np.repeat(a, K - 1, axis=1)
    e1 = np.tile(a, (1, K - 1))
    v0 = np.broadcast_to(c, e0.shape)
    cand = np.stack([v0, e0, e1], axis=-1)
    cand.sort(axis=-1)
    ck = cand[..., 0] * (P * P) + cand[..., 1] * P + cand[..., 2]
    cpos = np.clip(np.searchsorted(uk, ck.ravel()), 0, len(uk) - 1)
    gt_mask = (uk[cpos] == ck.ravel()).reshape(N, M)

    all_N_pred = gt_mask.sum(1).astype(np.float64)
    manifold = (all_N_gt * 2.0) == all_N_pred
    w = manifold.astype(np.float32)

    inv_denom = np.float32(1.0 / max(float(w.sum(dtype=np.float64)) * M, 1.0))
    inv_cnt = np.float32(1.0 / max(float(gm.sum(dtype=np.float64)), 1.0))
    return gt_mask, gm, w, inv_denom, inv_cnt


def _make_shards(x, gt_mask, gm, w, inv_denom, inv_cnt):
    """Build per-core input dicts. x is [N,256] f32."""
    N, M = x.shape
    rows_per_core = N // N_CORES          # 2048
    parts = 128
    rpp = rows_per_core // parts          # 16 rows per partition

    # masked-x values padded to L per row (L chosen from data, power-of-2-ish)
    mask_per_row = gt_mask.sum(1)
    L = max(8, int(mask_per_row.max()))
    L = int(2 ** np.ceil(np.log2(L)))
    rr, cc = np.nonzero(gt_mask)
    xm = np.zeros((N, L), np.float32)
    slot = np.zeros(N, np.int64)
    # vectorized bucket fill: rank of each nonzero within its row
    row_starts = np.zeros(N + 1, np.int64)
    np.add.at(row_starts, rr + 1, 1)
    row_starts = np.cumsum(row_starts)
    ranks = np.arange(len(rr)) - row_starts[rr]
    xm[rr, ranks] = x[rr, cc]

    # compacted gm groups, padded; distributed evenly over cores
    gn, gi = np.nonzero(gm)               # group ids (row, i)
    total = len(gn)
    per_core = int(np.ceil(total / N_CORES))
    g_chunks = int(np.ceil(per_core / parts))   # free-dim group chunks
    cap = g_chunks * parts                       # groups per core
    pl3 = x.reshape(N, 16, 16)

    pad_group = np.full(16, -B_PAD, np.float32)
    pad_group[0] = B_PAD
    pad_group[1] = B_PAD

    in_maps = []
    for core in range(N_CORES):
        r0, r1 = core * rows_per_core, (core + 1) * rows_per_core
        # transposed: m on partitions (2 x 128), rows on free dim
        xc = np.ascontiguousarray(x[r0:r1].T)                      # [256, 2048]
        xmc = np.ascontiguousarray(xm[r0:r1]).reshape(parts, rpp * L)
        wc = np.ascontiguousarray(w[r0:r1]).reshape(parts, rpp)
        w1p = np.ascontiguousarray(w[r0:r1]).reshape(1, rows_per_core)

        lo, hi = core * per_core, min((core + 1) * per_core, total)
        gsel = np.broadcast_to(pad_group, (cap, 16)).copy()
        if hi > lo:
            gsel[: hi - lo] = pl3[gn[lo:hi], gi[lo:hi], :]
        # layout [parts, g_chunks, 16]: group j -> partition j % parts
        gsel = np.ascontiguousarray(
            gsel.reshape(g_chunks, parts, 16).transpose(1, 0, 2)
        ).reshape(parts, g_chunks * 16)

        consts = np.broadcast_to(
            np.array([inv_denom, inv_cnt, -inv_denom], np.float32), (parts, 3)
        ).copy()
        in_maps.append({
            "x": xc, "xm": xmc, "wrow": wc, "w1p": w1p, "gsel": gsel,
            "consts": consts,
        })
    return in_maps, L, g_chunks


# ---------------------------------------------------------------- bass build
def _build_bass(L, g_chunks):
    from contextlib import ExitStack

    import concourse.bacc as bacc
    import concourse.bass as bass
    import concourse.mybir as mybir
    import concourse.tile as tile

    f32 = mybir.dt.float32
    AFT = mybir.ActivationFunctionType
    ALU = mybir.AluOpType
    AX = mybir.AxisListType

    parts, rpp, M = 128, 16, 256
    G = g_chunks

    R = 2048  # rows per core, free dim of transposed tiles

    nc = bacc.Bacc(
        "TRN2", target_bir_lowering=False, debug=False,
        enable_asserts=False, num_devices=N_CORES,
    )
    x_d = nc.dram_tensor("x", [2 * parts, R], f32, kind="ExternalInput").ap()
    xm_d = nc.dram_tensor("xm", [parts, rpp * L], f32, kind="ExternalInput").ap()
    w_d = nc.dram_tensor("wrow", [parts, rpp], f32, kind="ExternalInput").ap()
    w1_d = nc.dram_tensor("w1p", [1, R], f32, kind="ExternalInput").ap()
    g_d = nc.dram_tensor("gsel", [parts, G * 16], f32, kind="ExternalInput").ap()
    c_d = nc.dram_tensor("consts", [parts, 3], f32, kind="ExternalInput").ap()
    out_d = nc.dram_tensor("out", [1, 3], f32, kind="ExternalOutput").ap()

    bf16 = mybir.dt.bfloat16
    with tile.TileContext(nc) as tc, ExitStack() as ctx:
        pool = ctx.enter_context(tc.tile_pool(name="main", bufs=1))
        ps_pool = ctx.enter_context(tc.tile_pool(name="ps", bufs=1, space="PSUM"))

        # --- DMAs: x split into 8 column-chunks across queues; gsel first ---
        gt = pool.tile([parts, G * 16], f32)
        gh = G * 16 // 2
        nc.sync.dma_start(gt[:, :gh], g_d[:, :gh])
        nc.sync.dma_start(gt[:, gh:], g_d[:, gh:])
        halves = []
        DCH = 512
        for h in range(2):
            xth = pool.tile([parts, R], f32, name=f"xt{h}")
            for c in range(R // DCH):
                nc.sync.dma_start(
                    xth[:, c * DCH:(c + 1) * DCH],
                    x_d[h * parts:(h + 1) * parts, c * DCH:(c + 1) * DCH])
            halves.append(xth)
        xmt = pool.tile([parts, rpp * L], f32)
        nc.sync.dma_start(xmt[:], xm_d[:])
        wt = pool.tile([parts, rpp], f32)
        nc.sync.dma_start(wt[:], w_d[:])
        w1t = pool.tile([1, R], f32)
        nc.sync.dma_start(w1t[:], w1_d[:])
        ct = pool.tile([parts, 3], f32)
        nc.sync.dma_start(ct[:], c_d[:])

        onesw = pool.tile([parts, rpp], bf16)  # stationary ones for colsum
        nc.vector.memset(onesw[:], 1.0)

        # --- top-8 per compacted gm-group (exact, descending) ---
        top8 = pool.tile([parts, G * 8], f32)
        for g in range(G):
            nc.vector.max(top8[:, g * 8:(g + 1) * 8], gt[:, g * 16:(g + 1) * 16])
        t3 = top8[:].rearrange("p (g e) -> p g e", e=8)

        # --- softplus: Exp then Ln(e+1), chunked for pipelining; sp in bf16 ---
        ACH = 1024
        NAC = R // ACH
        exps, sps = [], []
        for h, xth in enumerate(halves):
            exps.append(pool.tile([parts, R], f32, name=f"e{h}"))
            sps.append(pool.tile([parts, R], bf16, name=f"sp{h}"))
        for c in range(NAC):
            sl = slice(c * ACH, (c + 1) * ACH)
            for h in range(2):
                nc.scalar.activation(exps[h][:, sl], halves[h][:, sl], AFT.Exp)
        pn_exp = pool.tile([parts, 2 * G], f32)
        pn_in = t3[:, :, 1:3]  # [128, G, 2] ranks 1 (pos) and 2 (neg)
        nc.scalar.activation(pn_exp[:].rearrange("p (g e) -> p g e", e=2),
                             pn_in, AFT.Exp)
        for c in range(NAC):
            sl = slice(c * ACH, (c + 1) * ACH)
            for h in range(2):
                nc.scalar.activation(sps[h][:, sl], exps[h][:, sl], AFT.Ln,
                                     bias=1.0)
        pn_ln = pool.tile([parts, 2 * G], f32)
        nc.scalar.activation(pn_ln[:], pn_exp[:], AFT.Ln, bias=1.0)

        # --- row sums of softplus: ones^T @ sp on PE (column sums), then
        #     fused (rowsp * w) accumulate straight off the PSUM row ---
        psum_cs = ps_pool.tile([rpp, R], f32)
        junk1p = pool.tile([1, R], f32)
        sttacc = pool.tile([1, 4], f32)
        NCH = 512
        for c in range(R // NCH):
            sl = slice(c * NCH, (c + 1) * NCH)
            for h in range(2):
                nc.tensor.matmul(psum_cs[:, sl], onesw[:], sps[h][:, sl],
                                 start=(h == 0), stop=(h == 1))
            nc.vector.scalar_tensor_tensor(
                out=junk1p[:, sl], in0=psum_cs[0:1, sl], scalar=1.0,
                in1=w1t[:, sl], op0=ALU.bypass, op1=ALU.mult,
                accum_out=sttacc[:, c:c + 1])
        wspsum = pool.tile([1, 1], f32)
        nc.vector.tensor_reduce(wspsum[:], sttacc[:], axis=AX.X, op=ALU.add)

        # --- masked-x weighted row sums (tiny, natural [128,16] layout) ---
        rowxm = pool.tile([parts, rpp], f32)
        nc.vector.tensor_reduce(
            rowxm[:], xmt[:].rearrange("p (r l) -> p r l", l=L),
            axis=AX.X, op=ALU.add,
        )
        wxm = pool.tile([parts, rpp], f32)
        nc.vector.tensor_tensor(wxm[:], rowxm[:], wt[:], op=ALU.mult)
        xmacc = pool.tile([parts, 1], f32)
        nc.vector.tensor_reduce(xmacc[:], wxm[:], axis=AX.X, op=ALU.add)

        # --- pos/neg partials: sp(-pos) = sp(pos) - pos; sp(neg) direct ---
        pn3 = pn_ln[:].rearrange("p (g e) -> p g e", e=2)
        possub = pool.tile([parts, G], f32)
        nc.vector.tensor_tensor(possub[:], pn3[:, :, 0], t3[:, :, 1],
                                op=ALU.subtract)
        posacc = pool.tile([parts, 1], f32)
        nc.vector.tensor_reduce(posacc[:], possub[:], axis=AX.X, op=ALU.add)
        negacc = pool.tile([parts, 1], f32)
        nc.vector.tensor_reduce(negacc[:], pn3[:, :, 1], axis=AX.X, op=ALU.add)

        # --- scale by host constants, pack, partition-reduce via PE ---
        # col0: -(sum_p w*rowxm)*inv_denom everywhere, +wspsum*inv_denom on p0
        parts_t = pool.tile([parts, 3], f32)
        nc.vector.tensor_tensor(parts_t[:, 0:1], xmacc[:], ct[:, 2:3], op=ALU.mult)
        extra = pool.tile([1, 1], f32)
        nc.vector.tensor_tensor(extra[:], wspsum[:], ct[0:1, 0:1], op=ALU.mult)
        nc.vector.tensor_tensor(parts_t[0:1, 0:1], parts_t[0:1, 0:1], extra[:],
                                op=ALU.add)
        nc.vector.tensor_tensor(parts_t[:, 1:2], posacc[:], ct[:, 1:2], op=ALU.mult)
        nc.vector.tensor_tensor(parts_t[:, 2:3], negacc[:], ct[:, 1:2], op=ALU.mult)

        ones = pool.tile([parts, 1], f32)
        nc.vector.memset(ones[:], 1.0)
        pst = ps_pool.tile([1, 3], f32)
        nc.tensor.matmul(pst[:], ones[:], parts_t[:], start=True, stop=True)
        sb_out = pool.tile([1, 3], f32)
        nc.scalar.copy(sb_out[:], pst[:])
        nc.sync.dma_start(out_d[:], sb_out[:])

    nc.compile()
    return nc


# ---------------------------------------------------------------- entrypoint
def _run(pred_logits, points, knn_indices, gt_triangles, **run_kwargs):
    from concourse.bass_utils import run_bass_kernel_spmd

    x = np.ascontiguousarray(np.asarray(pred_logits, dtype=np.float32))
    gt_mask, gm, w, inv_denom, inv_cnt = _host_prep(
        pred_logits, points, knn_indices, gt_triangles)
    in_maps, L, g_chunks = _make_shards(x, gt_mask, gm, w, inv_denom, inv_cnt)
    nc = _build_bass(L, g_chunks)
    res = run_bass_kernel_spmd(nc, in_maps, core_ids=list(range(N_CORES)),
                               **run_kwargs)
    total = np.zeros(3, np.float64)
    for r in res.results:
        total += np.asarray(r["out"], dtype=np.float64).reshape(3)
    return total.astype(np.float32), res


def kernel(pred_logits, points, knn_indices, gt_triangles):
    out, _ = _run(pred_logits, points, knn_indices, gt_triangles)
    return out


# revision 34
# speedup vs baseline: 2.0778x; 1.7641x over previous
"""Trainium2 Bass kernel for nn_BinaryLoss (BCE triangle-mesh loss).

Structure
---------
Host (integer combinatorics on the tiny index tensors only; no FP math on
logits): sorted-triangle key table -> unique keys; undirected GT edge set;
per-vertex unique-triangle counts; candidate-triple membership gt_mask
[N,256] via searchsorted; manifold row mask w [N]; edge mask gm [N,16].
Two exact identities drive the device plan:
  * gt_labels_masked == gt_mask (a GT triangle always contributes its own
    (e0,e1) edge to full_mat, so the dense adjacency lookup is redundant),
  * sum_m [sp(x) - x*mask] needs only softplus sums plus the sum of x over
    masked positions (<= 8 per row here, gathered to a narrow [rows,L]).
Only manifold rows (w==1, ~800 of 16384) contribute to the main loss, so
just those rows' logits ship to the device.

Device (all logit FP math, 8 cores data-parallel, per core):
  * gsel = compacted gm==1 groups of 16 logits, exp()'d on ScalarE
    (monotonic, ranks unchanged) then the DVE Max8 instruction gives the
    exact descending top-8 per group: rank1=exp(pos), rank2=exp(neg).
    sp(-pos)=Ln(1+1/exp(pos)) via DVE reciprocal + Ln, sp(neg)=Ln(1+exp(neg))
    -- no exp needed after the single activation-table switch.
  * selected rows: softplus via Exp then Ln(1+e) on ScalarE (transposed
    layout), per-partition sums on DVE.
  * ScalarE/DVE instruction orders are pinned with explicit dep edges to
    keep one exp->ln table transition and a stall-free DVE tail.
  * per-core raw partial sums [128,6] DMA out; the host applies inv_denom /
    inv_cnt and does the cross-core/partition scalar all-reduce.
Pad rows/groups use +-30 logits so their softplus terms are ~1e-13.
"""
import os
import numpy as np

N_CORES = 8
B_PAD = 30.0  # pad-group magnitude: softplus(-30) ~ 9e-14


# ---------------------------------------------------------------- host prep
def _host_prep(pred_logits, points, knn_indices, gt_triangles):
    N, K = knn_indices.shape
    M = (K - 1) * (K - 1)
    num_pts = points.shape[0]
    P = num_pts + 1

    tri = np.sort(np.asarray(gt_triangles, dtype=np.int64), axis=1)
    keys = tri[:, 0] * (P * P) + tri[:, 1] * P + tri[:, 2]
    uk = np.unique(keys)

    ut0, ut1, ut2 = uk // (P * P), (uk // P) % P, uk % P
    counts = np.zeros(P, np.float64)
    np.add.at(counts, ut0, 1.0)
    np.add.at(counts, ut1, (ut1 != ut0).astype(np.float64))
    np.add.at(counts, ut2, (ut2 != ut1).astype(np.float64))
    all_N_gt = counts[np.asarray(knn_indices[:, 0], dtype=np.int64)]

    e_u = np.concatenate([np.minimum(tri[:, 0], tri[:, 1]),
                          np.minimum(tri[:, 1], tri[:, 2]),
                          np.minimum(tri[:, 0], tri[:, 2])])
    e_v = np.concatenate([np.maximum(tri[:, 0], tri[:, 1]),
                          np.maximum(tri[:, 1], tri[:, 2]),
                          np.maximum(tri[:, 0], tri[:, 2])])
    ekeys = np.unique(e_u * P + e_v)

    c = np.asarray(knn_indices[:, 0], dtype=np.int64)[:, None]
    a = np.asarray(knn_indices[:, 1:], dtype=np.int64)
    q = np.minimum(c, a) * P + np.maximum(c, a)
    pos = np.clip(np.searchsorted(ekeys, q.ravel()), 0, len(ekeys) - 1)
    gm = (ekeys[pos] == q.ravel()).reshape(N, K - 1)

    e0 = np.repeat(a, K - 1, axis=1)
    e1 = np.tile(a, (1, K - 1))
    v0 = np.broadcast_to(c, e0.shape)
    cand = np.stack([v0, e0, e1], axis=-1)
    cand.sort(axis=-1)
    ck = cand[..., 0] * (P * P) + cand[..., 1] * P + cand[..., 2]
    cpos = np.clip(np.searchsorted(uk, ck.ravel()), 0, len(uk) - 1)
    gt_mask = (uk[cpos] == ck.ravel()).reshape(N, M)

    all_N_pred = gt_mask.sum(1).astype(np.float64)
    manifold = (all_N_gt * 2.0) == all_N_pred
    w = manifold.astype(np.float32)

    inv_denom = np.float32(1.0 / max(float(w.sum(dtype=np.float64)) * M, 1.0))
    inv_cnt = np.float32(1.0 / max(float(gm.sum(dtype=np.float64)), 1.0))
    return gt_mask, gm, w, inv_denom, inv_cnt


def _make_shards(x, gt_mask, gm, w, inv_denom, inv_cnt):
    """Build per-core input dicts. x is [N,256] f32."""
    N, M = x.shape
    parts = 128

    # masked-x values padded to L per row (L chosen from data)
    mask_per_row = gt_mask.sum(1)
    L = max(8, int(mask_per_row.max()))
    L = int(2 ** np.ceil(np.log2(L)))
    rr, cc = np.nonzero(gt_mask)
    xm = np.zeros((N, L), np.float32)
    row_starts = np.zeros(N + 1, np.int64)
    np.add.at(row_starts, rr + 1, 1)
    row_starts = np.cumsum(row_starts)
    ranks = np.arange(len(rr)) - row_starts[rr]
    xm[rr, ranks] = x[rr, cc]

    # only manifold rows (w==1) contribute to the main BCE: select them
    sel = np.nonzero(w)[0]
    W = len(sel)
    cap_pc = max(parts, int(np.ceil(W / (N_CORES * parts))) * parts)
    CAP = cap_pc * N_CORES
    xs = np.full((CAP, M), -B_PAD, np.float32)   # pad rows: softplus ~ 1e-13
    xs[:W] = x[sel]
    xms = np.zeros((CAP, L), np.float32)
    xms[:W] = xm[sel]

    # compacted gm groups, padded; distributed evenly over cores
    gn, gi = np.nonzero(gm)               # group ids (row, i)
    total = len(gn)
    per_core = int(np.ceil(total / N_CORES))
    g_chunks = max(1, int(np.ceil(per_core / parts)))  # free-dim group chunks
    cap = g_chunks * parts                       # groups per core
    pl3 = x.reshape(N, 16, 16)

    pad_group = np.full(16, -B_PAD, np.float32)
    pad_group[0] = B_PAD
    pad_group[1] = B_PAD

    in_maps = []
    for core in range(N_CORES):
        s0, s1 = core * cap_pc, (core + 1) * cap_pc
        xc = np.ascontiguousarray(xs[s0:s1].T)          # [256, cap_pc] f32
        kk = cap_pc // parts
        xmc = np.ascontiguousarray(xms[s0:s1]).reshape(parts, kk * L)

        lo, hi = core * per_core, min((core + 1) * per_core, total)
        gsel = np.broadcast_to(pad_group, (cap, 16)).copy()
        if hi > lo:
            gsel[: hi - lo] = pl3[gn[lo:hi], gi[lo:hi], :]
        gsel = np.ascontiguousarray(
            gsel.reshape(g_chunks, parts, 16).transpose(1, 0, 2)
        ).reshape(parts, g_chunks * 16)

        in_maps.append({"x": xc, "xm": xmc, "gsel": gsel})
    return in_maps, L, g_chunks, cap_pc


# ---------------------------------------------------------------- bass build
def _build_bass(L, g_chunks, cap_pc):
    from contextlib import ExitStack

    import concourse.bacc as bacc
    import concourse.mybir as mybir
    import concourse.tile as tile

    f32 = mybir.dt.float32
    bf16 = mybir.dt.bfloat16
    AFT = mybir.ActivationFunctionType
    ALU = mybir.AluOpType
    AX = mybir.AxisListType

    parts, rpp = 128, 16
    G = g_chunks
    S = cap_pc          # selected rows per core
    KK = S // parts     # xm row-chunks per partition

    nc = bacc.Bacc(
        "TRN2", target_bir_lowering=False, debug=False,
        enable_asserts=False, num_devices=N_CORES,
    )
    x_d = nc.dram_tensor("x", [2 * parts, S], f32, kind="ExternalInput").ap()
    xm_d = nc.dram_tensor("xm", [parts, KK * L], f32, kind="ExternalInput").ap()
    g_d = nc.dram_tensor("gsel", [parts, G * 16], f32, kind="ExternalInput").ap()
    out_d = nc.dram_tensor("out", [128, 6], f32, kind="ExternalOutput").ap()

    with tile.TileContext(nc) as tc, ExitStack() as ctx:
        from concourse.tile import add_dep_helper

        def chain(lst):
            for a, b in zip(lst, lst[1:]):
                add_dep_helper(b.ins, a.ins, sync=True, reason="engine order")

        pool = ctx.enter_context(tc.tile_pool(name="main", bufs=1))

        acts = []  # explicit ScalarE program order (avoids table-load thrash)
        dves = []  # pinned DVE order for the post-Max8 tail

        # hoist the exp-table load: dummy activation with no DMA deps
        dumt = pool.tile([1, 8], f32)
        nc.vector.memset(dumt[:], 0.0)
        dumo = pool.tile([1, 8], f32)
        acts.append(nc.scalar.activation(dumo[:], dumt[:], AFT.Exp))

        # --- DMAs: gsel first (feeds the critical Max8 chain) ---
        gt = pool.tile([parts, G * 16], f32)
        NGC = 4
        gch = G * 16 // NGC
        for i in range(NGC):
            nc.sync.dma_start(gt[:, i * gch:(i + 1) * gch],
                              g_d[:, i * gch:(i + 1) * gch])
        halves = []
        for h in range(2):
            xth = pool.tile([parts, S], f32, name=f"xt{h}", tag=f"xt{h}")
            nc.gpsimd.dma_start(xth[:], x_d[h * parts:(h + 1) * parts, :])
            halves.append(xth)
        xmt = pool.tile([parts, KK * L], f32)
        nc.gpsimd.dma_start(xmt[:], xm_d[:])

        # --- exp over gsel (monotonic: Max8 ranks unchanged) interleaved
        #     with the selected-row exps so the ln table switch lands early
        ge = pool.tile([parts, G * 16], f32)
        ets, sps = [], []
        for h in range(2):
            ets.append(pool.tile([parts, S], f32, name=f"e{h}", tag=f"e{h}"))
            sps.append(pool.tile([parts, S], f32, name=f"sp{h}", tag=f"sp{h}"))
        for i in range(2):
            acts.append(nc.scalar.activation(ge[:, i * gch:(i + 1) * gch],
                                             gt[:, i * gch:(i + 1) * gch],
                                             AFT.Exp))
        acts.append(nc.scalar.activation(ets[0][:], halves[0][:], AFT.Exp))
        for i in range(2, NGC):
            acts.append(nc.scalar.activation(ge[:, i * gch:(i + 1) * gch],
                                             gt[:, i * gch:(i + 1) * gch],
                                             AFT.Exp))
        acts.append(nc.scalar.activation(ets[1][:], halves[1][:], AFT.Exp))

        # --- top-8 per compacted gm-group on exp-domain values ---
        top8 = pool.tile([parts, G * 8], f32)
        for g in range(G):
            nc.vector.max(top8[:, g * 8:(g + 1) * 8],
                          ge[:, g * 16:(g + 1) * 16])
        t8e = top8[:].rearrange("p (g e) -> p g e", e=8)
        # pn_cat = [1/exp(pos) , exp(neg)] -> Ln(1+.) gives sp(-pos), sp(neg)
        pn_cat = pool.tile([parts, 2 * G], f32)
        dves.append(nc.vector.reciprocal(pn_cat[:, :G], t8e[:, :, 1]))
        dves.append(nc.vector.tensor_copy(pn_cat[:, G:], t8e[:, :, 2]))

        # --- ln-set phase ---
        for h in range(2):
            acts.append(nc.scalar.activation(sps[h][:], ets[h][:], AFT.Ln,
                                             bias=1.0))
        pn_ln = pool.tile([parts, 2 * G], f32)
        acts.append(nc.scalar.activation(pn_ln[:], pn_cat[:], AFT.Ln, bias=1.0))

        # --- raw partial sums into accs columns; host applies the scales ---
        accs = pool.tile([parts, 6], f32)
        dves.append(nc.vector.tensor_reduce(
            accs[:, 2:3], xmt[:].rearrange("p (k l) -> p k l", l=L),
            axis=AX.XY, op=ALU.add))
        dves.append(nc.vector.tensor_reduce(accs[:, 3:4], sps[0][:],
                                            axis=AX.X, op=ALU.add))
        dves.append(nc.vector.tensor_reduce(accs[:, 4:5], sps[1][:],
                                            axis=AX.X, op=ALU.add))
        dves.append(nc.vector.tensor_reduce(accs[:, 0:1], pn_ln[:, :G],
                                            axis=AX.X, op=ALU.add))
        dves.append(nc.vector.tensor_reduce(accs[:, 1:2], pn_ln[:, G:],
                                            axis=AX.X, op=ALU.add))
        dves.append(nc.vector.memset(accs[:, 5:6], 0.0))
        nc.sync.dma_start(out_d[:], accs[:])

        # pin ScalarE program order: all exp-set work, then all ln-set work
        chain(acts)
        chain(dves)

    if os.environ.get('ATL_PATCH', '0') == '1':
        _prefer_combined_act_table()
    nc.compile()
    return nc


_ACT_PATCHED = False


def _prefer_combined_act_table():
    """Bias bacc's table chooser toward the set holding both Exp and Ln so a
    single ACT_TABLE_LOAD serves the whole kernel."""
    global _ACT_PATCHED
    if _ACT_PATCHED:
        return
    import concourse.bacc as bacc_mod
    import concourse.hw_specs as hw_specs_mod

    orig = hw_specs_mod.get_activation_tables

    def _patched(arch):
        tabs = orig(arch)
        pref = "natural_log_exp_and_others"
        if pref in tabs:
            out = {pref: tabs[pref]}
            out.update({k: v for k, v in tabs.items() if k != pref})
            return out
        return tabs

    bacc_mod.get_activation_tables = _patched
    _ACT_PATCHED = True


# ---------------------------------------------------------------- entrypoint
def _run(pred_logits, points, knn_indices, gt_triangles, **run_kwargs):
    from concourse.bass_utils import run_bass_kernel_spmd

    x = np.ascontiguousarray(np.asarray(pred_logits, dtype=np.float32))
    gt_mask, gm, w, inv_denom, inv_cnt = _host_prep(
        pred_logits, points, knn_indices, gt_triangles)
    in_maps, L, g_chunks, cap_pc = _make_shards(x, gt_mask, gm, w,
                                                 inv_denom, inv_cnt)
    nc = _build_bass(L, g_chunks, cap_pc)
    res = run_bass_kernel_spmd(nc, in_maps, core_ids=list(range(N_CORES)),
                               **run_kwargs)
    acc = np.zeros(6, np.float64)
    for r in res.results:
        acc += np.asarray(r["out"], dtype=np.float64).reshape(128, 6).sum(axis=0)
    pos_t, neg_t, xm_t = acc[0], acc[1], acc[2]
    sp_t = acc[3] + acc[4]
    total = np.array([(sp_t - xm_t) * float(inv_denom),
                      pos_t * float(inv_cnt),
                      neg_t * float(inv_cnt)])
    return total.astype(np.float32), res


def kernel(pred_logits, points, knn_indices, gt_triangles):
    out, _ = _run(pred_logits, points, knn_indices, gt_triangles)
    return out
